# revision 30
# baseline (speedup 1.0000x reference)
"""Trainium2 Bass kernel for single-head attention:
    q = x @ W0; k = x @ W1; v = x @ W2
    out = softmax(q k^T / sqrt(O)) @ v
Shapes (full): x [16, 2048, 512], kernel [3, 512, 512] -> out [16, 2048, 512].
Sharding: data-parallel over batch, 2 batches per core on 8 NeuronCores.

Precision: scores are rank-1 dominated (W entries have mean 0.5, so
S ~= 128*sigma_i*sigma_j +- O(1e2) pre-scale). All tensors are stored
fp16 (10-bit mantissa); every matmul chain is a single fp16 pass
(q/k/v = fp16(x) @ fp16(W), scores q*k, AV = P16 * v16) with fp32 PSUM
accumulation. Numpy-emulated rel err 4.1e-3 against the 2e-2 gate
(fp16 emulation has matched HW exactly on previous variants).
"""

import math

import numpy as np

# Full-problem shapes (hardcoded per harness contract).
B_FULL = 16
N = 2048
D = 512
O = 512
N_CORES = 8
B_CORE = B_FULL // N_CORES  # 2 batches per core

NT = N // 128  # 16 row tiles
DT = D // 128  # 4 contraction tiles
OT = O // 128  # 4 o tiles
JB = N // 512  # 4 column blocks of 512
SCALE = 1.0 / math.sqrt(float(O))

_CACHE = {}


def _build_program(loop_n=None):
    import contextlib

    import concourse.mybir as mybir
    import concourse.tile as tile
    from concourse import bacc
    from concourse.masks import make_identity

    f32 = mybir.dt.float32
    f16 = mybir.dt.float16
    AX = mybir.AxisListType.X
    EXP = mybir.ActivationFunctionType.Exp
    COPY = mybir.ActivationFunctionType.Copy

    nc = bacc.Bacc("TRN2", target_bir_lowering=False, debug=False,
                   num_devices=N_CORES)
    x = nc.dram_tensor("x", [B_CORE, N, D], f32, kind="ExternalInput").ap()
    w = nc.dram_tensor("kernel", [3, D, O], f32, kind="ExternalInput").ap()
    out = nc.dram_tensor("out", [B_CORE, N, O], f32, kind="ExternalOutput").ap()

    with tile.TileContext(nc) as tc:
        with (
            tc.tile_pool(name="persist", bufs=1) as pers,
            tc.tile_pool(name="sp", bufs=4, space="PSUM") as sp,      # 4 banks
            tc.tile_pool(name="ptp", bufs=2, space="PSUM") as ptp,    # 1 bank
            tc.tile_pool(name="avp", bufs=1, space="PSUM") as avp,    # 2 banks
            tc.tile_pool(name="xs", bufs=6) as xs,
            tc.tile_pool(name="pp", bufs=2) as pp,
            tc.tile_pool(name="ptsb", bufs=2) as ptsb,
            tc.tile_pool(name="osb", bufs=3) as osb,
            tc.tile_pool(name="sm", bufs=4) as sm,
        ):
            # Persistent SBUF tensors (distinct tags -> one slot each).
            ident = pers.tile([128, 128], f32, tag="ident", name="ident")
            idb = pers.tile([128, 128], f16, tag="idb", name="idb")
            # fp16 weights, [d, o] layout
            wsp = {}
            for wi, wn in ((0, "wq"), (1, "wk"), (2, "wv")):
                wsp[wn] = pers.tile([128, DT, O], f16, tag=wn, name=wn)
            xh = pers.tile([128, DT, N], f16, tag="xh", name="xh")   # x^T
            qT = pers.tile([128, OT, N], f16, tag="qT", name="qT")   # q^T
            kT = pers.tile([128, OT, N], f16, tag="kT", name="kT")   # k^T
            vv = pers.tile([128, NT, O], f16, tag="vv", name="vv")   # v [n, o]

            make_identity(nc, ident)
            nc.gpsimd.memset(idb, 0.0)
            nc.gpsimd.affine_select(
                out=idb, in_=idb,
                compare_op=mybir.AluOpType.not_equal,
                fill=1.0, base=0, pattern=[[-1, 128]], channel_multiplier=1)
            # load weights as fp16
            for wi, wn in ((0, "wq"), (1, "wk"), (2, "wv")):
                wh = wsp[wn]
                for dt in range(DT):
                    wf = xs.tile([128, O], f32, tag="xs", name="wstage")
                    nc.sync.dma_start(
                        out=wf, in_=w[wi, dt * 128:(dt + 1) * 128, :])
                    nc.scalar.activation(out=wh[:, dt, :], in_=wf, func=COPY)

            loop_ctx = (tc.For_i(0, loop_n, 1) if loop_n
                        else contextlib.nullcontext())
            with loop_ctx:
                for b in range(B_CORE):
                    # ---- Phase A: load x, PE-transpose, split hi/lo ----
                    for nb in range(JB):
                        stage = []
                        for k in range(4):
                            nt = nb * 4 + k
                            st = xs.tile([128, D], f32, tag="xs")
                            nc.sync.dma_start(
                                out=st, in_=x[b, nt * 128:(nt + 1) * 128, :])
                            stage.append(st)
                        for dt in range(DT):
                            pack = sp.tile([128, 512], f32, tag="ps")
                            for k in range(4):
                                nc.tensor.transpose(
                                    pack[:, k * 128:(k + 1) * 128],
                                    stage[k][:, dt * 128:(dt + 1) * 128],
                                    ident)
                            nsl = slice(nb * 512, (nb + 1) * 512)
                            nc.scalar.activation(
                                out=xh[:, dt, nsl], in_=pack, func=COPY)

                    # ---- Phase B: projections ----
                    # q^T/k^T [o, n]: lhsT = W[d, o-tile], rhs = x^T[d, n-blk]
                    # The JB n-blocks accumulate in 4 round-robin PSUM banks
                    # (4 concurrently-open groups) so consecutive matmuls
                    # never target the same bank: a same-bank accumulation
                    # chain serializes fill with drain (~350 ns/MM vs
                    # ~222 ns/MM rotated), and consecutive matmuls share the
                    # same stationary weight tile.
                    for wn, dst in (("wq", qT), ("wk", kT)):
                        wh = wsp[wn]
                        for ot in range(OT):
                            osl = slice(ot * 128, (ot + 1) * 128)
                            pss = [sp.tile([128, 512], f32, tag="ps",
                                           name=f"pjps{nb}")
                                   for nb in range(JB)]
                            for dt in range(DT):
                                for nb in range(JB):
                                    nsl = slice(nb * 512, (nb + 1) * 512)
                                    nc.tensor.matmul(
                                        pss[nb],
                                        lhsT=wh[:, dt, osl],
                                        rhs=xh[:, dt, nsl],
                                        start=(dt == 0),
                                        stop=(dt == DT - 1))
                            for nb in range(JB):
                                nsl = slice(nb * 512, (nb + 1) * 512)
                                nc.scalar.activation(
                                    out=dst[:, ot, nsl], in_=pss[nb],
                                    func=COPY)
                    # v[n, o]: lhsT = x^T[d, n-tile], rhs = Wv[d, :], 1 pass
                    wvh = wsp["wv"]
                    for nt in range(NT):
                        ps = sp.tile([128, 512], f32, tag="ps")
                        for dt in range(DT):
                            nc.tensor.matmul(
                                ps,
                                lhsT=xh[:, dt, nt * 128:(nt + 1) * 128],
                                rhs=wvh[:, dt, :],
                                start=(dt == 0), stop=(dt == DT - 1))
                        nc.scalar.activation(out=vv[:, nt, :], in_=ps, func=COPY)

                    # ---- Phase C: attention, one 128-row q tile at a time.
                    # Emission is software-pipelined: PT/AV of tile i-1 are
                    # emitted interleaved with S/softmax of tile i so the
                    # in-order PE queue never waits on exp(i).
                    def emit_pt(prev):
                        p_t, rr, it = prev
                        pt_t = ptsb.tile([128, N], f16, tag="pt")
                        for g in range(JB):
                            pk = ptp.tile([128, 512], f16, tag="ptp")
                            for k2 in range(4):
                                jt = g * 4 + k2
                                nc.tensor.transpose(
                                    pk[:, k2 * 128:(k2 + 1) * 128],
                                    p_t[:, jt * 128:(jt + 1) * 128],
                                    idb)
                            nc.vector.tensor_copy(
                                out=pt_t[:, g * 512:(g + 1) * 512], in_=pk)
                        return pt_t

                    def emit_av(prev, pt_t):
                        p_t, rr, it = prev
                        # Two PSUM banks, alternating in 4-MM chunks, so the
                        # same-bank accumulation gap stays >= 4 matmuls
                        # (avoids the fill/drain serialization of a single
                        # 16-MM chain into one bank).
                        # Two closed 8-MM groups in separate banks halve
                        # the same-bank accumulation chain (which serializes
                        # fill with drain). Combine with per-bank scaling on
                        # the scalar engine (one PSUM operand per instruction)
                        # and a DVE add of the SBUF copies.
                        oa = avp.tile([128, 512], f32, tag="av_a")
                        ob2 = avp.tile([128, 512], f32, tag="av_b")
                        for cc, acc in ((0, oa), (1, ob2)):
                            for j2 in range(8):
                                jt = cc * 8 + j2
                                nc.tensor.matmul(
                                    acc,
                                    lhsT=pt_t[:, jt * 128:(jt + 1) * 128],
                                    rhs=vv[:, jt, :],
                                    start=(j2 == 0), stop=(j2 == 7))
                        s1 = osb.tile([128, 512], f32, tag="osum")
                        s2 = osb.tile([128, 512], f32, tag="osum2")
                        nc.scalar.activation(
                            out=s1, in_=oa, func=COPY, scale=rr[:, 7:8])
                        nc.scalar.activation(
                            out=s2, in_=ob2, func=COPY, scale=rr[:, 7:8])
                        ot_sb = osb.tile([128, 512], f32, tag="o")
                        nc.vector.tensor_add(ot_sb, s1, s2)
                        nc.sync.dma_start(
                            out=out[b, it * 128:(it + 1) * 128, :], in_=ot_sb)

                    prev = None
                    for it in range(NT):
                        isl = slice(it * 128, (it + 1) * 128)
                        pt_prev = emit_pt(prev) if prev is not None else None
                        p_t = pp.tile([128, N], f16, tag="p")
                        mx = sm.tile([128, 8], f32, tag="mx")
                        rr = sm.tile([128, 8], f32, tag="rr")
                        s_banks = []
                        for jb in range(JB):
                            jsl = slice(jb * 512, (jb + 1) * 512)
                            ps = sp.tile([128, 512], f32, tag="ps")
                            for ot in range(OT):
                                nc.tensor.matmul(
                                    ps,
                                    lhsT=qT[:, ot, isl],
                                    rhs=kT[:, ot, jsl],
                                    start=(ot == 0), stop=(ot == OT - 1))
                            nc.vector.reduce_max(
                                out=mx[:, jb:jb + 1], in_=ps, axis=AX)
                            s_banks.append(ps)
                        nc.vector.tensor_max(mx[:, 4:5], mx[:, 0:1], mx[:, 1:2])
                        nc.vector.tensor_max(mx[:, 5:6], mx[:, 2:3], mx[:, 3:4])
                        nc.vector.tensor_max(mx[:, 6:7], mx[:, 4:5], mx[:, 5:6])
                        # bias = -max(scaled scores)
                        nc.vector.tensor_scalar_mul(mx[:, 7:8], mx[:, 6:7], -SCALE)
                        # exp(i) is emitted BEFORE emit_av(i-1) so it sits
                        # ahead of the AV drain-scales in the scalar FIFO:
                        # it only needs mx(i), so it runs concurrently with
                        # the AV matmuls on the PE, and p_t(i) is ready when
                        # the next tile's PT transposes start.
                        for jb in range(JB):
                            nc.scalar.activation(
                                out=p_t[:, jb * 512:(jb + 1) * 512],
                                in_=s_banks[jb],
                                func=EXP,
                                bias=mx[:, 7:8],
                                scale=SCALE,
                                accum_out=rr[:, jb:jb + 1])
                        nc.vector.tensor_add(rr[:, 4:5], rr[:, 0:1], rr[:, 1:2])
                        nc.vector.tensor_add(rr[:, 5:6], rr[:, 2:3], rr[:, 3:4])
                        nc.vector.tensor_add(rr[:, 6:7], rr[:, 4:5], rr[:, 5:6])
                        nc.vector.reciprocal(rr[:, 7:8], rr[:, 6:7])
                        if prev is not None:
                            emit_av(prev, pt_prev)
                        prev = (p_t, rr, it)
                    pt_prev = emit_pt(prev)
                    emit_av(prev, pt_prev)

    nc.compile()
    return nc


def _get_nc():
    if "nc" not in _CACHE:
        _CACHE["nc"] = _build_program()
    return _CACHE["nc"]


def _make_runner(nc):
    """Persistent jitted SPMD executor over the 8 axon NeuronCores.

    Mirrors concourse.bass2jax.run_bass_via_pjrt's multi-core path, but
    caches the jitted callable so repeated kernel() calls don't re-trace.
    """
    import jax
    import jax.numpy as jnp
    from jax.sharding import Mesh, PartitionSpec
    from jax.experimental.shard_map import shard_map
    import concourse.mybir as mybir
    from concourse import bass2jax

    bass2jax.install_neuronx_cc_hook()

    partition_name = (nc.partition_id_tensor.name
                      if nc.partition_id_tensor else None)
    in_names = []
    out_names = []
    out_avals = []
    for alloc in nc.m.functions[0].allocations:
        if not isinstance(alloc, mybir.MemoryLocationSet):
            continue
        name = alloc.memorylocations[0].name
        if alloc.kind == "ExternalInput":
            if name != partition_name:
                in_names.append(name)
        elif alloc.kind == "ExternalOutput":
            out_names.append(name)
            out_avals.append(
                jax.core.ShapedArray(tuple(alloc.tensor_shape),
                                     mybir.dt.np(alloc.dtype)))
    n_params = len(in_names)
    all_in_names = tuple(in_names) + tuple(out_names)
    if partition_name is not None:
        all_in_names = all_in_names + (partition_name,)

    def _body(*args):
        operands = list(args)
        if partition_name is not None:
            operands.append(bass2jax.partition_id_tensor())
        outs = bass2jax._bass_exec_p.bind(
            *operands,
            out_avals=tuple(out_avals),
            in_names=all_in_names,
            out_names=tuple(out_names),
            lowering_input_output_aliases=(),
            sim_require_finite=True,
            sim_require_nnan=True,
            nc=nc,
        )
        return tuple(outs)

    devices = jax.devices()[:N_CORES]
    mesh = Mesh(np.asarray(devices), ("core",))
    n_outs = len(out_names)
    sharded = jax.jit(
        shard_map(_body, mesh=mesh,
                  in_specs=(PartitionSpec("core"),) * (n_params + n_outs),
                  out_specs=(PartitionSpec("core"),) * n_outs,
                  check_rep=False),
        keep_unused=True,
    )

    zero_shapes = [(N_CORES * a.shape[0],) + a.shape[1:] for a in out_avals]
    zero_dtypes = [a.dtype for a in out_avals]

    @jax.jit
    def make_zeros():
        return tuple(jnp.zeros(s, d) for s, d in zip(zero_shapes, zero_dtypes))

    return (sharded, tuple(in_names), tuple(out_names), out_avals, make_zeros)


def _get_runner():
    if "runner" not in _CACHE:
        _CACHE["runner"] = _make_runner(_get_nc())
    return _CACHE["runner"]


def _run_global(runner, global_ins):
    """global_ins: dict name -> np/jax array with axis0 = concat over cores."""
    sharded, in_names, out_names, out_avals, make_zeros = runner
    args = [global_ins[n] for n in in_names]
    outs = sharded(*args, *make_zeros())
    return dict(zip(out_names, outs))


def _rep_w(w):
    return np.broadcast_to(w, (N_CORES,) + w.shape).reshape(
        N_CORES * w.shape[0], *w.shape[1:])


def kernel(x: np.ndarray, kernel: np.ndarray) -> np.ndarray:
    x = np.ascontiguousarray(x, dtype=np.float32)
    w = np.ascontiguousarray(kernel, dtype=np.float32)
    outs = _run_global(_get_runner(), {"x": x, "kernel": _rep_w(w)})
    out = np.asarray(outs["out"])
    return out.reshape(B_FULL, N, O)
